# revision 29
# baseline (speedup 1.0000x reference)
"""Trainium2 Bass kernel for nn_Attention_5153960755626.

Multi-head attention (B=1, N=4096, C=768, H=12, D=64) distributed over 8
NeuronCores, sequence-parallel: core i computes attention output rows
[i*512, (i+1)*512).  Full K / V are exchanged with AllGather collectives.

v2 layout strategy (zero on-chip transposes):
  - host passes xT [C, N-slice] (fp32r-rounded) and qkv_wT [C, 3C]
    (fp32r-rounded, q columns pre-scaled by sqrt(D) to fold the
    reference's q/scale quirk)
  - qT,kT computed transposed [d', n] via fp32r matmuls, rounded to plain
    bf16 (error budget analysis: bf16 rounding of q and k gives logit
    error ~8e-3 std -> ~0.8% softmax weight error -> well inside the 2e-2
    rel-err budget); V computed natural [n, d'] in bf16 with a ones
    column per head so the PV matmul also produces the softmax denom
  - logits computed transposed [keys, q] with a SINGLE bf16 matmul per
    128-key chunk (K=64 contraction: same instruction cost as K=128 on
    the PE -- cost is output-columns only)
  - exp on ScalarE straight from PSUM, no max subtraction (max |logit|
    ~55 for this distribution, safe in fp32)
  - PV matmul contracts keys on partitions (bf16), softmax normalization
    via DVE reciprocal + DMA partition-broadcast
  - projection uses outT [c', n] directly as lhsT (bf16)

Pipelining: logits of group g+1 are emitted before PV of group g so the
TensorE stream never stalls waiting for ScalarE's exp.  A tiny dummy
AllGather is issued first so the one-time collective bootstrap/skew
barrier overlaps the QKV phase instead of stalling the k0 gather.
"""

import os
import sys

sys.path.insert(0, "/opt/trn_rl_repo")

import numpy as np
import ml_dtypes

from contextlib import ExitStack

from concourse import bass, bacc, tile, mybir
from concourse.bass_utils import run_bass_kernel_spmd

NCORES = 8
N = 4096          # sequence length
C = 768           # channels
H = 12            # heads
DH = 64           # head dim
NL = N // NCORES  # local sequence rows per core (512)
CCH = C // 128    # channel chunks (6)
MC = N // 128     # key chunks over full sequence (32)
VW = 65           # per-head V width incl. ones column
SCALE = float(DH) ** 0.5  # reference divides q by D**-0.5 => q * 8

f32 = mybir.dt.float32
f32r = mybir.dt.float32r
bf16 = mybir.dt.bfloat16
Exp = mybir.ActivationFunctionType.Exp
Identity = mybir.ActivationFunctionType.Identity
MUL = mybir.AluOpType.mult
ADD = mybir.AluOpType.add
SUB = mybir.AluOpType.subtract


def _build_program():
    nc = bacc.Bacc(
        "TRN2",
        target_bir_lowering=False,
        debug=False,
        enable_asserts=False,
        num_devices=NCORES,
    )

    xT_d = nc.dram_tensor("xT", [C, NL], f32r, kind="ExternalInput").ap()
    qw_d = nc.dram_tensor("qkv_wT", [C, 3 * C], f32r, kind="ExternalInput").ap()
    bqk_d = nc.dram_tensor("qkv_b_qk", [128, 2 * CCH], f32, kind="ExternalInput").ap()
    bv_d = nc.dram_tensor("qkv_b_v", [1, C], f32, kind="ExternalInput").ap()
    pw_d = nc.dram_tensor("proj_wT", [C, C], bf16, kind="ExternalInput").ap()
    pb_d = nc.dram_tensor("proj_b", [1, C], f32, kind="ExternalInput").ap()
    out_d = nc.dram_tensor("out", [NL, C], f32, kind="ExternalOutput").ap()

    groups = [list(range(NCORES))]

    with tile.TileContext(nc) as tc, ExitStack() as es:
        persist = es.enter_context(tc.tile_pool(name="persist", bufs=1))
        dram = es.enter_context(tc.tile_pool(name="dram", bufs=1, space="DRAM"))

        # ---- persistent SBUF ----
        qh2 = [persist.tile([DH, NL], bf16, tag=f"qh2_{h}", name=f"qh2_{h}") for h in range(H)]
        outT = [persist.tile([128, NL], bf16, tag=f"outT{m}", name=f"outT{m}") for m in range(CCH)]
        bqk = persist.tile([128, 2 * CCH], f32, tag="bqk", name="bqk")
        vbc = persist.tile([128, C], f32, tag="vbc", name="vbc")
        pbc = persist.tile([128, C], f32, tag="pbc", name="pbc")
        projw = [persist.tile([128, C], bf16, tag=f"projw{m}", name=f"projw{m}") for m in range(CCH)]

        # ---- collective buffers: ONE merged k+v AllGather per head pair.
        # The CC stream executes ops serially at ~15-20us each nearly
        # independent of size, so fewer/bigger ops shorten the chain.
        KSZ = 2 * DH * NL            # k part: [2, 64, 512]
        VJ = 128 * 2 * VW            # one v row-block: [128, 130]
        VSZ = (NL // 128) * VJ       # v part: [4, 128, 130]
        # Shard packing over 6 AllGather ops.  The CC stream serializes ops
        # at ~15-25us each and cannot start before its ~66us bootstrap, so
        # op0 is kept SMALL (k0 + first half of v0) to get pair-0 logits
        # going earliest; op1 carries the rest of v0 plus k1+v1; ops 2-5
        # carry k_t+v_t.  Attention consumes key-chunks j-major so pair 0
        # only needs v0's j0/j1 blocks for its first 16 chunks.
        #   region0: [k0 | v0j0 | v0j1]
        #   region1: [v0j2 | v0j3 | k1 | v1(j0..j3)]
        #   region2-5: [k_t | v_t(j0..j3)]
        R0SZ = KSZ + 2 * VJ
        R1SZ = 2 * VJ + KSZ + VSZ
        RSZ = KSZ + VSZ
        REG_OFF = [0, R0SZ, R0SZ + R1SZ] + [R0SZ + R1SZ + RSZ * i for i in range(1, 4)]
        REG_SZ = [R0SZ, R1SZ, RSZ, RSZ, RSZ, RSZ]
        TOT = REG_OFF[-1] + RSZ
        # absolute offsets of k_t and v_t[j] in the packed shard
        K_OFF = [0, REG_OFF[1] + 2 * VJ] + [REG_OFF[i] for i in range(2, 6)]
        V_OFF = [
            [REG_OFF[0] + KSZ, REG_OFF[0] + KSZ + VJ, REG_OFF[1], REG_OFF[1] + VJ],
            [REG_OFF[1] + 2 * VJ + KSZ + j * VJ for j in range(4)],
        ] + [[REG_OFF[i] + KSZ + j * VJ for j in range(4)] for i in range(2, 6)]
        # which AG op delivers k_t / v_t[j]
        K_OP = [0, 1, 2, 3, 4, 5]
        V_OP = [[0, 0, 1, 1], [1] * 4] + [[i] * 4 for i in range(2, 6)]

        kvshard = dram.tile([TOT], bf16, tag="kvshard", name="kvshard")
        kvall = [
            dram.tile(
                [NCORES, REG_SZ[i]], bf16, tag=f"kvall{i}", name=f"kvall{i}", addr_space="Shared"
            )
            for i in range(CCH)
        ]

        def kshard_view(t, hh):
            o = K_OFF[t] + hh * DH * NL
            return kvshard[o : o + DH * NL].rearrange("(p n) -> p n", n=NL)

        def allgather(src_t, dst_t):
            nc.gpsimd.collective_compute(
                "AllGather",
                mybir.AluOpType.bypass,
                replica_groups=groups,
                ins=[src_t.opt()],
                outs=[dst_t.opt()],
            )

        # ================= Phase 1: QKV projection =================
        with (
            tc.tile_pool(name="w1", bufs=1) as w1,
            tc.tile_pool(name="p1", bufs=3, space="PSUM") as p1,
            tc.tile_pool(name="sc1", bufs=2) as sc1,
            tc.tile_pool(name="kv1", bufs=1) as kv1,
        ):
            xts = [w1.tile([128, NL], f32r, tag=f"xts{c}", name=f"xts{c}") for c in range(CCH)]
            qwk = [w1.tile([128, C], f32r, tag=f"qwk{c}", name=f"qwk{c}") for c in range(CCH)]
            qwv = [w1.tile([128, C], f32r, tag=f"qwv{c}", name=f"qwv{c}") for c in range(CCH)]
            qwq = [w1.tile([128, C], f32r, tag=f"qwq{c}", name=f"qwq{c}") for c in range(CCH)]
            # load order: x + k-columns first, interleaved per chunk so the
            # first qk_psum matmul can start after ~2 chunks have landed
            for c in range(CCH):
                nc.sync.dma_start(xts[c][:], xT_d[c * 128 : (c + 1) * 128, :])
                nc.sync.dma_start(qwk[c][:], qw_d[c * 128 : (c + 1) * 128, C : 2 * C])
            nc.sync.dma_start(bqk[:], bqk_d[:])
            for c in range(CCH):
                nc.sync.dma_start(qwv[c][:], qw_d[c * 128 : (c + 1) * 128, 2 * C : 3 * C])
            nc.sync.dma_start(vbc[:], bv_d[0:1, :].to_broadcast((128, C)))

            def qk_psum(wtiles, m):
                ps = p1.tile([128, NL], f32, tag="p1qk", name="p1qk")
                for c in range(CCH):
                    nc.tensor.matmul(
                        ps[:],
                        lhsT=wtiles[c][:, m * 128 : (m + 1) * 128],
                        rhs=xts[c][:],
                        start=(c == 0),
                        stop=(c == CCH - 1),
                    )
                return ps

            # bias-add + bf16 cast fused on the (otherwise idle) ScalarE.
            def emit_k(t):
                ps = qk_psum(qwk, t)
                kh = sc1.tile([128, NL], bf16, tag="khs", name="khs")
                nc.scalar.activation(kh[:], ps[:], Identity, bias=bqk[:, CCH + t : CCH + t + 1])
                for hh in range(2):
                    nc.sync.dma_start(kshard_view(t, hh), kh[hh * 64 : hh * 64 + 64, :])

            # ---- k0 first (it gates the first AllGather), then v pairs
            # 0-2 (half 0), so AG(kv0) can fire right as the CC bootstrap
            # barrier completes; remaining k chunks + v half 1 feed the
            # rest of the AG chain.
            emit_k(0)

            # ---- v natural layout [n, d'] bf16 with ones columns; one tile
            # for all row-blocks (col j*780 + t*130), half-outer loop so
            # pairs 0-2 complete before pairs 3-5
            JW = H * VW  # 780 cols per row-block
            vloc = kv1.tile([128, 4 * JW], bf16, tag="vloc", name="vloc")
            nc.vector.memset(vloc[:], 1.0)

            def emit_v_j(half, j):
                ps = p1.tile([128, 384], f32, tag="p1v", name="p1v")
                for c in range(CCH):
                    nc.tensor.matmul(
                        ps[:],
                        lhsT=xts[c][:, j * 128 : (j + 1) * 128],
                        rhs=qwv[c][:, half * 384 : (half + 1) * 384],
                        start=(c == 0),
                        stop=(c == CCH - 1),
                    )
                dst = vloc[:, j * JW : (j + 1) * JW].rearrange(
                    "p (h e) -> p h e", e=VW
                )[:, half * 6 : (half + 1) * 6, 0:DH]
                vsrc_ = ps[:].rearrange("p (h e) -> p h e", e=DH)
                bias = vbc[:, half * 384 : (half + 1) * 384].rearrange(
                    "p (h e) -> p h e", e=DH
                )
                nc.vector.tensor_tensor(dst, vsrc_, bias, ADD)

            def emit_v_half(half):
                for j in range(NL // 128):
                    emit_v_j(half, j)

            def write_v(t, j0, nj):
                # one DMA: v_t row-blocks j0..j0+nj-1 -> packed shard region
                dst = kvshard[V_OFF[t][j0] : V_OFF[t][j0] + nj * VJ].rearrange(
                    "(j p w) -> p j w", p=128, w=2 * VW
                )
                src = vloc[:].rearrange("p (j x) -> p j x", j=4)[
                    :, j0 : j0 + nj, 2 * t * VW : 2 * (t + 1) * VW
                ]
                nc.sync.dma_start(dst, src)

            def ag(i):
                allgather(kvshard[REG_OFF[i] : REG_OFF[i] + REG_SZ[i]], kvall[i])

            emit_v_j(0, 0)
            emit_v_j(0, 1)
            write_v(0, 0, 2)
            ag(0)
            # q weights loaded only now: their HBM traffic would otherwise
            # delay the slowest core's k0+v0j01 arming of the first AG (the
            # CC barrier waits for all 8 cores)
            for c in range(CCH):
                nc.sync.dma_start(qwq[c][:], qw_d[c * 128 : (c + 1) * 128, 0:C])
            emit_v_j(0, 2)
            emit_v_j(0, 3)
            emit_k(1)
            write_v(0, 2, 2)
            write_v(1, 0, 4)
            ag(1)
            emit_k(2)
            write_v(2, 0, 4)
            ag(2)
            emit_v_half(1)
            emit_k(3)
            write_v(3, 0, 4)
            ag(3)
            emit_k(4)
            write_v(4, 0, 4)
            ag(4)
            emit_k(5)
            write_v(5, 0, 4)
            ag(5)

            # ---- q chunks: plain bf16 (no hi/lo split; single-pass logits)
            for t in range(CCH):
                ps = qk_psum(qwq, t)
                qh = sc1.tile([128, NL], bf16, tag="qhs", name="qhs")
                nc.scalar.activation(qh[:], ps[:], Identity, bias=bqk[:, t : t + 1])
                for hh in range(2):
                    h = 2 * t + hh
                    nc.sync.dma_start(qh2[h][:], qh[hh * 64 : hh * 64 + 64, :])

        # ================= Phase 2: attention =================
        with (
            tc.tile_pool(name="attn", bufs=2) as at,
            tc.tile_pool(name="lp", bufs=3, space="PSUM") as lpool,
            tc.tile_pool(name="pvp", bufs=2, space="PSUM") as pvpool,
            tc.tile_pool(name="ep", bufs=4) as epool,
            tc.tile_pool(name="np", bufs=2) as npool,
        ):
            GRPS = [2] * 16  # 32 key-chunks per head

            # flat pipelined schedule: emit logits+exp of unit u, then PV of
            # unit u-1, so TensorE never stalls waiting for ScalarE's exp
            vp_tiles = {}
            pv_tiles = {}
            kp_tiles = {}

            def emit_logits(u):
                t, hh, gi, mc0, g = u
                h = 2 * t + hh
                if hh == 0 and gi == 0:
                    # kp loads first: logits consume k a step before PV
                    # consumes v
                    krel = K_OFF[t] - REG_OFF[K_OP[t]]
                    for hh2 in range(2):
                        kp2 = at.tile([DH, N], bf16, tag="kp", name="kp", bufs=4)
                        nc.sync.dma_start(
                            kp2[:].rearrange("p (b n) -> p b n", b=NCORES),
                            kvall[K_OP[t]][
                                :, krel + hh2 * DH * NL : krel + (hh2 + 1) * DH * NL
                            ].rearrange("b (p n) -> p b n", n=NL),
                        )
                        kp_tiles[2 * t + hh2] = kp2
                    # vp columns j-major (j*8+b) so each per-j load DMA is a
                    # contiguous column range (clean subtile deps: pair 0's
                    # j0/j1 arrive in AG op0, j2/j3 in op1)
                    vp = at.tile([128, MC * 2 * VW], bf16, tag="vpair", name="vpair")
                    for j in range(NL // 128):
                        vrel = V_OFF[t][j] - REG_OFF[V_OP[t][j]]
                        nc.sync.dma_start(
                            vp[
                                :, j * NCORES * 2 * VW : (j + 1) * NCORES * 2 * VW
                            ].rearrange("p (b w) -> p b w", w=2 * VW),
                            kvall[V_OP[t][j]][:, vrel : vrel + VJ].rearrange(
                                "b (p w) -> p b w", w=2 * VW
                            ),
                        )
                    vp_tiles[t] = vp
                if gi == 0:
                    pv_tiles[h] = pvpool.tile([VW, NL], f32, tag="pv", name="pv")
                kp = kp_tiles[h]
                lp = lpool.tile([128, 2 * NL], f32, tag="lg", name="lg")
                for i in range(g):
                    s = mc0 + i  # j-major chunk order: s = j*8 + b
                    jj, b = s // NCORES, s % NCORES
                    o = lp[:, i * NL : (i + 1) * NL]
                    w = kp[:, b * NL + jj * 128 : b * NL + (jj + 1) * 128]
                    nc.tensor.matmul(o, lhsT=w, rhs=qh2[h][:], start=True, stop=True)
                et = epool.tile([128, 2 * NL], bf16, tag="et", name="et")
                nc.scalar.activation(et[:, : g * NL], lp[:, : g * NL], Exp)
                return et

            def emit_pv(u, et):
                t, hh, gi, mc0, g = u
                h = 2 * t + hh
                vp = vp_tiles[t]
                pv = pv_tiles[h]
                for i in range(g):
                    s = mc0 + i
                    nc.tensor.matmul(
                        pv[:],
                        lhsT=vp[:, s * 2 * VW + hh * VW : s * 2 * VW + hh * VW + VW],
                        rhs=et[:, i * NL : (i + 1) * NL],
                        start=(s == 0),
                        stop=(s == MC - 1),
                    )
                if mc0 + g == MC:
                    # end of head: normalize (broadcast 1/denom via the idle
                    # GpSimd engine instead of a DRAM round-trip)
                    rec = npool.tile([1, NL], f32, tag="rec", name="rec")
                    nc.vector.reciprocal(rec[:], pv[DH : DH + 1, :])
                    rbc = npool.tile([64, NL], f32, tag="rbc", name="rbc")
                    nc.gpsimd.partition_broadcast(rbc[:], rec[:], channels=64)
                    nc.vector.tensor_tensor(
                        outT[t][hh * 64 : hh * 64 + 64, :], pv[0:DH, :], rbc[:], MUL
                    )

            units = []
            for t in range(CCH):
                for hh in range(2):
                    mc0 = 0
                    for gi, g in enumerate(GRPS):
                        units.append((t, hh, gi, mc0, g))
                        mc0 += g

            # PV lags TWO units behind logits: L(u+1) then sits AHEAD of the
            # act(u)-gated P(u-1) in the TensorE queue, so the act engine's
            # input is always ready the moment act(u) retires (act is the
            # loop's critical path at ~1.6us/unit).
            pending = []
            for u in units:
                et = emit_logits(u)
                pending.append((u, et))
                if len(pending) > 2:
                    emit_pv(*pending.pop(0))
                # late loads (needed only in phase 3), emitted after pair-0's
                # kp/vp DMAs so they don't delay the attention-loop start
                if u[0] == 0 and u[1] == 1 and u[2] == 0:
                    nc.sync.dma_start(pbc[:], pb_d[0:1, :].to_broadcast((128, C)))
                    for m in range(CCH):
                        nc.sync.dma_start(projw[m][:], pw_d[m * 128 : (m + 1) * 128, :])
            for pu in pending:
                emit_pv(*pu)

        # ================= Phase 3: projection =================
        with (
            tc.tile_pool(name="pp", bufs=2, space="PSUM") as ppool,
            tc.tile_pool(name="po", bufs=2) as opool,
        ):
            for j in range(NL // 128):
                for half in range(2):
                    ps = ppool.tile([128, 384], f32, tag="pp", name="pp")
                    for m in range(CCH):
                        nc.tensor.matmul(
                            ps[:],
                            lhsT=outT[m][:, j * 128 : (j + 1) * 128],
                            rhs=projw[m][:, half * 384 : (half + 1) * 384],
                            start=(m == 0),
                            stop=(m == CCH - 1),
                        )
                    osb = opool.tile([128, 384], f32, tag="osb", name="osb")
                    nc.vector.tensor_tensor(
                        osb[:], ps[:], pbc[:, half * 384 : (half + 1) * 384], ADD
                    )
                    nc.sync.dma_start(
                        out_d[j * 128 : (j + 1) * 128, half * 384 : (half + 1) * 384],
                        osb[:],
                    )

    nc.compile()
    return nc


_PROGRAM = None


def _get_program():
    global _PROGRAM
    if _PROGRAM is None:
        _PROGRAM = _build_program()
    return _PROGRAM


def _round_fp32r(a):
    """Round fp32 to the fp32r bit format: 11-bit mantissa (RNE), low 12 bits zero."""
    u = np.ascontiguousarray(a, dtype=np.float32).view(np.uint32)
    lsb = (u >> 12) & 1
    u = (u + 0x7FF + lsb) & 0xFFFFF000
    return u.view(np.float32)


def _host_prep(x, qkv_w, qkv_b, proj_w, proj_b):
    x2 = np.asarray(x, dtype=np.float32).reshape(N, C)
    xT = _round_fp32r(np.ascontiguousarray(x2.T))  # [C, N]
    qkv_wT = np.ascontiguousarray(np.asarray(qkv_w, dtype=np.float32).T).copy()
    qkv_wT[:, :C] *= SCALE  # fold the q/scale quirk into W_q
    qkv_wT = _round_fp32r(qkv_wT)
    bqk = np.asarray(qkv_b, dtype=np.float32)[: 2 * C].reshape(2 * CCH, 128).T.copy()
    bqk[:, :CCH] *= SCALE  # fold scale into q bias too
    bv = np.asarray(qkv_b, dtype=np.float32)[2 * C :].reshape(1, C).copy()
    pwT = np.ascontiguousarray(np.asarray(proj_w, dtype=np.float32).T).astype(
        ml_dtypes.bfloat16
    )
    pb = np.asarray(proj_b, dtype=np.float32).reshape(1, C).copy()

    in_maps = []
    for i in range(NCORES):
        in_maps.append(
            {
                "xT": np.ascontiguousarray(xT[:, i * NL : (i + 1) * NL]),
                "qkv_wT": qkv_wT,
                "qkv_b_qk": bqk,
                "qkv_b_v": bv,
                "proj_wT": pwT,
                "proj_b": pb,
            }
        )
    return in_maps


def kernel(x, qkv_w, qkv_b, proj_w, proj_b):
    nc = _get_program()
    in_maps = _host_prep(x, qkv_w, qkv_b, proj_w, proj_b)
    kw = {}
    if os.environ.get("KERNEL_TRACE_DIR"):
        kw["tmpdir"] = os.environ["KERNEL_TRACE_DIR"]
    res = run_bass_kernel_spmd(
        nc,
        in_maps,
        core_ids=list(range(NCORES)),
        trace=bool(int(os.environ.get("KERNEL_TRACE", "0"))),
        **kw,
    )
    if res.exec_time_ns is not None:
        print(f"HW exec time: {res.exec_time_ns} ns", file=sys.stderr)
    out = np.concatenate(
        [np.asarray(res.results[i]["out"]) for i in range(NCORES)], axis=0
    )
    return out.reshape(1, N, C).astype(np.float32)


if __name__ == "__main__":
    rng = np.random.default_rng(0)
    x = rng.standard_normal((1, N, C), dtype=np.float32)
    qkv_w = (rng.standard_normal((3 * C, C)) * 0.01).astype(np.float32)
    qkv_b = np.zeros((3 * C,), np.float32)
    proj_w = (rng.standard_normal((C, C)) * 0.01).astype(np.float32)
    proj_b = np.zeros((C,), np.float32)
    out = kernel(x=x, qkv_w=qkv_w, qkv_b=qkv_b, proj_w=proj_w, proj_b=proj_b)
    print(out.shape, out.dtype)


# revision 30
# speedup vs baseline: 1.0317x; 1.0317x over previous
"""Trainium2 Bass kernel for nn_Attention_5153960755626.

Multi-head attention (B=1, N=4096, C=768, H=12, D=64) distributed over 8
NeuronCores, sequence-parallel: core i computes attention output rows
[i*512, (i+1)*512).  Full K / V are exchanged with AllGather collectives.

v2 layout strategy (zero on-chip transposes):
  - host passes xT [C, N-slice] (fp32r-rounded) and qkv_wT [C, 3C]
    (fp32r-rounded, q columns pre-scaled by sqrt(D) to fold the
    reference's q/scale quirk)
  - qT,kT computed transposed [d', n] via fp32r matmuls, rounded to plain
    bf16 (error budget analysis: bf16 rounding of q and k gives logit
    error ~8e-3 std -> ~0.8% softmax weight error -> well inside the 2e-2
    rel-err budget); V computed natural [n, d'] in bf16 with a ones
    column per head so the PV matmul also produces the softmax denom
  - logits computed transposed [keys, q] with a SINGLE bf16 matmul per
    128-key chunk (K=64 contraction: same instruction cost as K=128 on
    the PE -- cost is output-columns only)
  - exp on ScalarE straight from PSUM, no max subtraction (max |logit|
    ~55 for this distribution, safe in fp32)
  - PV matmul contracts keys on partitions (bf16), softmax normalization
    via DVE reciprocal + DMA partition-broadcast
  - projection uses outT [c', n] directly as lhsT (bf16)

Pipelining: logits of group g+1 are emitted before PV of group g so the
TensorE stream never stalls waiting for ScalarE's exp.  A tiny dummy
AllGather is issued first so the one-time collective bootstrap/skew
barrier overlaps the QKV phase instead of stalling the k0 gather.
"""

import os
import sys

sys.path.insert(0, "/opt/trn_rl_repo")

import numpy as np
import ml_dtypes

from contextlib import ExitStack

from concourse import bass, bacc, tile, mybir
from concourse.bass_utils import run_bass_kernel_spmd

NCORES = 8
N = 4096          # sequence length
C = 768           # channels
H = 12            # heads
DH = 64           # head dim
NL = N // NCORES  # local sequence rows per core (512)
CCH = C // 128    # channel chunks (6)
MC = N // 128     # key chunks over full sequence (32)
VW = 65           # per-head V width incl. ones column
SCALE = float(DH) ** 0.5  # reference divides q by D**-0.5 => q * 8

f32 = mybir.dt.float32
f32r = mybir.dt.float32r
bf16 = mybir.dt.bfloat16
Exp = mybir.ActivationFunctionType.Exp
Identity = mybir.ActivationFunctionType.Identity
MUL = mybir.AluOpType.mult
ADD = mybir.AluOpType.add
SUB = mybir.AluOpType.subtract


def _build_program():
    nc = bacc.Bacc(
        "TRN2",
        target_bir_lowering=False,
        debug=False,
        enable_asserts=False,
        num_devices=NCORES,
    )

    xT_d = nc.dram_tensor("xT", [C, NL], f32r, kind="ExternalInput").ap()
    qw_d = nc.dram_tensor("qkv_wT", [C, 3 * C], f32r, kind="ExternalInput").ap()
    bqk_d = nc.dram_tensor("qkv_b_qk", [128, 2 * CCH], f32, kind="ExternalInput").ap()
    bv_d = nc.dram_tensor("qkv_b_v", [1, C], f32, kind="ExternalInput").ap()
    pw_d = nc.dram_tensor("proj_wT", [C, C], bf16, kind="ExternalInput").ap()
    pb_d = nc.dram_tensor("proj_b", [1, C], f32, kind="ExternalInput").ap()
    out_d = nc.dram_tensor("out", [NL, C], f32, kind="ExternalOutput").ap()

    groups = [list(range(NCORES))]

    with tile.TileContext(nc) as tc, ExitStack() as es:
        persist = es.enter_context(tc.tile_pool(name="persist", bufs=1))
        dram = es.enter_context(tc.tile_pool(name="dram", bufs=1, space="DRAM"))

        # ---- persistent SBUF ----
        qh2 = [persist.tile([DH, NL], bf16, tag=f"qh2_{h}", name=f"qh2_{h}") for h in range(H)]
        outT = [persist.tile([128, NL], bf16, tag=f"outT{m}", name=f"outT{m}") for m in range(CCH)]
        bqk = persist.tile([128, 2 * CCH], f32, tag="bqk", name="bqk")
        vbc = persist.tile([128, C], f32, tag="vbc", name="vbc")
        pbc = persist.tile([128, C], f32, tag="pbc", name="pbc")
        projw = [persist.tile([128, C], bf16, tag=f"projw{m}", name=f"projw{m}") for m in range(CCH)]

        # ---- collective buffers: ONE merged k+v AllGather per head pair.
        # The CC stream executes ops serially at ~15-20us each nearly
        # independent of size, so fewer/bigger ops shorten the chain.
        KSZ = 2 * DH * NL            # k part: [2, 64, 512]
        VJ = 128 * 2 * VW            # one v row-block: [128, 130]
        VSZ = (NL // 128) * VJ       # v part: [4, 128, 130]
        # Shard packing over 6 AllGather ops.  The CC stream serializes ops
        # at ~15-25us each and cannot start before its ~66us bootstrap, so
        # op0 is kept SMALL (k0 + first half of v0) to get pair-0 logits
        # going earliest; op1 carries the rest of v0 plus k1+v1; ops 2-5
        # carry k_t+v_t.  Attention consumes key-chunks j-major so pair 0
        # only needs v0's j0/j1 blocks for its first 16 chunks.
        #   region0: [k0 | v0j0 | v0j1]
        #   region1: [v0j2 | v0j3 | k1 | v1(j0..j3)]
        #   region2-5: [k_t | v_t(j0..j3)]
        R0SZ = KSZ + 2 * VJ
        R1SZ = 2 * VJ + KSZ + VSZ
        RSZ = KSZ + VSZ
        REG_OFF = [0, R0SZ, R0SZ + R1SZ] + [R0SZ + R1SZ + RSZ * i for i in range(1, 4)]
        REG_SZ = [R0SZ, R1SZ, RSZ, RSZ, RSZ, RSZ]
        TOT = REG_OFF[-1] + RSZ
        # absolute offsets of k_t and v_t[j] in the packed shard
        K_OFF = [0, REG_OFF[1] + 2 * VJ] + [REG_OFF[i] for i in range(2, 6)]
        V_OFF = [
            [REG_OFF[0] + KSZ, REG_OFF[0] + KSZ + VJ, REG_OFF[1], REG_OFF[1] + VJ],
            [REG_OFF[1] + 2 * VJ + KSZ + j * VJ for j in range(4)],
        ] + [[REG_OFF[i] + KSZ + j * VJ for j in range(4)] for i in range(2, 6)]
        # which AG op delivers k_t / v_t[j]
        K_OP = [0, 1, 2, 3, 4, 5]
        V_OP = [[0, 0, 1, 1], [1] * 4] + [[i] * 4 for i in range(2, 6)]

        kvshard = dram.tile([TOT], bf16, tag="kvshard", name="kvshard")
        kvall = [
            dram.tile(
                [NCORES, REG_SZ[i]], bf16, tag=f"kvall{i}", name=f"kvall{i}", addr_space="Shared"
            )
            for i in range(CCH)
        ]

        def kshard_view(t, hh):
            o = K_OFF[t] + hh * DH * NL
            return kvshard[o : o + DH * NL].rearrange("(p n) -> p n", n=NL)

        def allgather(src_t, dst_t):
            nc.gpsimd.collective_compute(
                "AllGather",
                mybir.AluOpType.bypass,
                replica_groups=groups,
                ins=[src_t.opt()],
                outs=[dst_t.opt()],
            )

        # ================= Phase 1: QKV projection =================
        with (
            tc.tile_pool(name="w1", bufs=1) as w1,
            tc.tile_pool(name="p1", bufs=3, space="PSUM") as p1,
            tc.tile_pool(name="sc1", bufs=2) as sc1,
            tc.tile_pool(name="kv1", bufs=1) as kv1,
        ):
            xts = [w1.tile([128, NL], f32r, tag=f"xts{c}", name=f"xts{c}") for c in range(CCH)]
            qwk = [w1.tile([128, C], f32r, tag=f"qwk{c}", name=f"qwk{c}") for c in range(CCH)]
            qwv = [w1.tile([128, C], f32r, tag=f"qwv{c}", name=f"qwv{c}") for c in range(CCH)]
            qwq = [w1.tile([128, C], f32r, tag=f"qwq{c}", name=f"qwq{c}") for c in range(CCH)]
            # load order: x + k-columns first, interleaved per chunk so the
            # first qk_psum matmul can start after ~2 chunks have landed
            for c in range(CCH):
                nc.sync.dma_start(xts[c][:], xT_d[c * 128 : (c + 1) * 128, :])
                nc.sync.dma_start(qwk[c][:], qw_d[c * 128 : (c + 1) * 128, C : 2 * C])
            nc.sync.dma_start(bqk[:], bqk_d[:])
            for c in range(CCH):
                nc.sync.dma_start(qwv[c][:], qw_d[c * 128 : (c + 1) * 128, 2 * C : 3 * C])
            nc.sync.dma_start(vbc[:], bv_d[0:1, :].to_broadcast((128, C)))

            def qk_psum(wtiles, m):
                ps = p1.tile([128, NL], f32, tag="p1qk", name="p1qk")
                for c in range(CCH):
                    nc.tensor.matmul(
                        ps[:],
                        lhsT=wtiles[c][:, m * 128 : (m + 1) * 128],
                        rhs=xts[c][:],
                        start=(c == 0),
                        stop=(c == CCH - 1),
                    )
                return ps

            # bias-add + bf16 cast fused on the (otherwise idle) ScalarE.
            def emit_k(t):
                ps = qk_psum(qwk, t)
                kh = sc1.tile([128, NL], bf16, tag="khs", name="khs")
                nc.scalar.activation(kh[:], ps[:], Identity, bias=bqk[:, CCH + t : CCH + t + 1])
                for hh in range(2):
                    nc.sync.dma_start(kshard_view(t, hh), kh[hh * 64 : hh * 64 + 64, :])

            # ---- k0 first (it gates the first AllGather), then v pairs
            # 0-2 (half 0), so AG(kv0) can fire right as the CC bootstrap
            # barrier completes; remaining k chunks + v half 1 feed the
            # rest of the AG chain.
            emit_k(0)

            # ---- v natural layout [n, d'] bf16 with ones columns; one tile
            # for all row-blocks (col j*780 + t*130), half-outer loop so
            # pairs 0-2 complete before pairs 3-5
            JW = H * VW  # 780 cols per row-block
            vloc = kv1.tile([128, 4 * JW], bf16, tag="vloc", name="vloc")
            nc.vector.memset(vloc[:], 1.0)

            def emit_v_j(half, j):
                ps = p1.tile([128, 384], f32, tag="p1v", name="p1v")
                for c in range(CCH):
                    nc.tensor.matmul(
                        ps[:],
                        lhsT=xts[c][:, j * 128 : (j + 1) * 128],
                        rhs=qwv[c][:, half * 384 : (half + 1) * 384],
                        start=(c == 0),
                        stop=(c == CCH - 1),
                    )
                dst = vloc[:, j * JW : (j + 1) * JW].rearrange(
                    "p (h e) -> p h e", e=VW
                )[:, half * 6 : (half + 1) * 6, 0:DH]
                vsrc_ = ps[:].rearrange("p (h e) -> p h e", e=DH)
                bias = vbc[:, half * 384 : (half + 1) * 384].rearrange(
                    "p (h e) -> p h e", e=DH
                )
                nc.vector.tensor_tensor(dst, vsrc_, bias, ADD)

            def emit_v_half(half):
                for j in range(NL // 128):
                    emit_v_j(half, j)

            def write_v(t, j0, nj):
                # one DMA: v_t row-blocks j0..j0+nj-1 -> packed shard region
                dst = kvshard[V_OFF[t][j0] : V_OFF[t][j0] + nj * VJ].rearrange(
                    "(j p w) -> p j w", p=128, w=2 * VW
                )
                src = vloc[:].rearrange("p (j x) -> p j x", j=4)[
                    :, j0 : j0 + nj, 2 * t * VW : 2 * (t + 1) * VW
                ]
                nc.sync.dma_start(dst, src)

            def ag(i):
                allgather(kvshard[REG_OFF[i] : REG_OFF[i] + REG_SZ[i]], kvall[i])

            emit_v_j(0, 0)
            emit_v_j(0, 1)
            write_v(0, 0, 2)
            ag(0)
            # q weights loaded only now: their HBM traffic would otherwise
            # delay the slowest core's k0+v0j01 arming of the first AG (the
            # CC barrier waits for all 8 cores)
            for c in range(CCH):
                nc.sync.dma_start(qwq[c][:], qw_d[c * 128 : (c + 1) * 128, 0:C])
            emit_v_j(0, 2)
            emit_v_j(0, 3)
            emit_k(1)
            write_v(0, 2, 2)
            write_v(1, 0, 4)
            ag(1)
            emit_k(2)
            write_v(2, 0, 4)
            ag(2)
            emit_v_half(1)
            emit_k(3)
            write_v(3, 0, 4)
            ag(3)
            emit_k(4)
            write_v(4, 0, 4)
            ag(4)
            emit_k(5)
            write_v(5, 0, 4)
            ag(5)

            # ---- q chunks: plain bf16 (no hi/lo split; single-pass logits)
            for t in range(CCH):
                ps = qk_psum(qwq, t)
                qh = sc1.tile([128, NL], bf16, tag="qhs", name="qhs")
                nc.scalar.activation(qh[:], ps[:], Identity, bias=bqk[:, t : t + 1])
                for hh in range(2):
                    h = 2 * t + hh
                    nc.sync.dma_start(qh2[h][:], qh[hh * 64 : hh * 64 + 64, :])

        # ================= Phase 2: attention =================
        with (
            tc.tile_pool(name="attn", bufs=2) as at,
            tc.tile_pool(name="lp", bufs=2, space="PSUM") as lpool,
            tc.tile_pool(name="pvp", bufs=2, space="PSUM") as pvpool,
            tc.tile_pool(name="ep", bufs=4) as epool,
            tc.tile_pool(name="np", bufs=2) as npool,
        ):
            GRPS = [3] * 10 + [2]  # 32 key-chunks per head

            # flat pipelined schedule: emit logits+exp of unit u, then PV of
            # unit u-1, so TensorE never stalls waiting for ScalarE's exp
            vp_tiles = {}
            pv_tiles = {}
            kp_tiles = {}

            def emit_logits(u):
                t, hh, gi, mc0, g = u
                h = 2 * t + hh
                if hh == 0 and gi == 0:
                    # kp loads first: logits consume k a step before PV
                    # consumes v
                    krel = K_OFF[t] - REG_OFF[K_OP[t]]
                    for hh2 in range(2):
                        kp2 = at.tile([DH, N], bf16, tag="kp", name="kp", bufs=4)
                        nc.sync.dma_start(
                            kp2[:].rearrange("p (b n) -> p b n", b=NCORES),
                            kvall[K_OP[t]][
                                :, krel + hh2 * DH * NL : krel + (hh2 + 1) * DH * NL
                            ].rearrange("b (p n) -> p b n", n=NL),
                        )
                        kp_tiles[2 * t + hh2] = kp2
                    # vp columns j-major (j*8+b) so each per-j load DMA is a
                    # contiguous column range (clean subtile deps: pair 0's
                    # j0/j1 arrive in AG op0, j2/j3 in op1)
                    vp = at.tile([128, MC * 2 * VW], bf16, tag="vpair", name="vpair")
                    for j in range(NL // 128):
                        vrel = V_OFF[t][j] - REG_OFF[V_OP[t][j]]
                        nc.sync.dma_start(
                            vp[
                                :, j * NCORES * 2 * VW : (j + 1) * NCORES * 2 * VW
                            ].rearrange("p (b w) -> p b w", w=2 * VW),
                            kvall[V_OP[t][j]][:, vrel : vrel + VJ].rearrange(
                                "b (p w) -> p b w", w=2 * VW
                            ),
                        )
                    vp_tiles[t] = vp
                if gi == 0:
                    pv_tiles[h] = pvpool.tile([VW, NL], f32, tag="pv", name="pv")
                kp = kp_tiles[h]
                lp = lpool.tile([128, 3 * NL], f32, tag="lg", name="lg")
                for i in range(g):
                    s = mc0 + i  # j-major chunk order: s = j*8 + b
                    jj, b = s // NCORES, s % NCORES
                    o = lp[:, i * NL : (i + 1) * NL]
                    w = kp[:, b * NL + jj * 128 : b * NL + (jj + 1) * 128]
                    nc.tensor.matmul(o, lhsT=w, rhs=qh2[h][:], start=True, stop=True)
                et = epool.tile([128, 3 * NL], bf16, tag="et", name="et")
                nc.scalar.activation(et[:, : g * NL], lp[:, : g * NL], Exp)
                return et

            def emit_pv(u, et):
                t, hh, gi, mc0, g = u
                h = 2 * t + hh
                vp = vp_tiles[t]
                pv = pv_tiles[h]
                for i in range(g):
                    s = mc0 + i
                    nc.tensor.matmul(
                        pv[:],
                        lhsT=vp[:, s * 2 * VW + hh * VW : s * 2 * VW + hh * VW + VW],
                        rhs=et[:, i * NL : (i + 1) * NL],
                        start=(s == 0),
                        stop=(s == MC - 1),
                    )
                if mc0 + g == MC:
                    # end of head: normalize (broadcast 1/denom via the idle
                    # GpSimd engine instead of a DRAM round-trip)
                    rec = npool.tile([1, NL], f32, tag="rec", name="rec")
                    nc.vector.reciprocal(rec[:], pv[DH : DH + 1, :])
                    rbc = npool.tile([64, NL], f32, tag="rbc", name="rbc")
                    nc.gpsimd.partition_broadcast(rbc[:], rec[:], channels=64)
                    nc.vector.tensor_tensor(
                        outT[t][hh * 64 : hh * 64 + 64, :], pv[0:DH, :], rbc[:], MUL
                    )

            units = []
            for t in range(CCH):
                for hh in range(2):
                    mc0 = 0
                    for gi, g in enumerate(GRPS):
                        units.append((t, hh, gi, mc0, g))
                        mc0 += g

            # PV lags TWO units behind logits: L(u+1) then sits AHEAD of the
            # act(u)-gated P(u-1) in the TensorE queue, so the act engine's
            # input is always ready the moment act(u) retires (act is the
            # loop's critical path at ~1.6us/unit).
            pending = []
            for u in units:
                et = emit_logits(u)
                pending.append((u, et))
                if len(pending) > 2:
                    emit_pv(*pending.pop(0))
                # late loads (needed only in phase 3), emitted after pair-0's
                # kp/vp DMAs so they don't delay the attention-loop start
                if u[0] == 0 and u[1] == 1 and u[2] == 0:
                    nc.sync.dma_start(pbc[:], pb_d[0:1, :].to_broadcast((128, C)))
                    for m in range(CCH):
                        nc.sync.dma_start(projw[m][:], pw_d[m * 128 : (m + 1) * 128, :])
            for pu in pending:
                emit_pv(*pu)

        # ================= Phase 3: projection =================
        with (
            tc.tile_pool(name="pp", bufs=2, space="PSUM") as ppool,
            tc.tile_pool(name="po", bufs=2) as opool,
        ):
            for j in range(NL // 128):
                for half in range(2):
                    ps = ppool.tile([128, 384], f32, tag="pp", name="pp")
                    for m in range(CCH):
                        nc.tensor.matmul(
                            ps[:],
                            lhsT=outT[m][:, j * 128 : (j + 1) * 128],
                            rhs=projw[m][:, half * 384 : (half + 1) * 384],
                            start=(m == 0),
                            stop=(m == CCH - 1),
                        )
                    osb = opool.tile([128, 384], f32, tag="osb", name="osb")
                    nc.vector.tensor_tensor(
                        osb[:], ps[:], pbc[:, half * 384 : (half + 1) * 384], ADD
                    )
                    nc.sync.dma_start(
                        out_d[j * 128 : (j + 1) * 128, half * 384 : (half + 1) * 384],
                        osb[:],
                    )

    nc.compile()
    return nc


_PROGRAM = None


def _get_program():
    global _PROGRAM
    if _PROGRAM is None:
        _PROGRAM = _build_program()
    return _PROGRAM


def _round_fp32r(a):
    """Round fp32 to the fp32r bit format: 11-bit mantissa (RNE), low 12 bits zero."""
    u = np.ascontiguousarray(a, dtype=np.float32).view(np.uint32)
    lsb = (u >> 12) & 1
    u = (u + 0x7FF + lsb) & 0xFFFFF000
    return u.view(np.float32)


def _host_prep(x, qkv_w, qkv_b, proj_w, proj_b):
    x2 = np.asarray(x, dtype=np.float32).reshape(N, C)
    xT = _round_fp32r(np.ascontiguousarray(x2.T))  # [C, N]
    qkv_wT = np.ascontiguousarray(np.asarray(qkv_w, dtype=np.float32).T).copy()
    qkv_wT[:, :C] *= SCALE  # fold the q/scale quirk into W_q
    qkv_wT = _round_fp32r(qkv_wT)
    bqk = np.asarray(qkv_b, dtype=np.float32)[: 2 * C].reshape(2 * CCH, 128).T.copy()
    bqk[:, :CCH] *= SCALE  # fold scale into q bias too
    bv = np.asarray(qkv_b, dtype=np.float32)[2 * C :].reshape(1, C).copy()
    pwT = np.ascontiguousarray(np.asarray(proj_w, dtype=np.float32).T).astype(
        ml_dtypes.bfloat16
    )
    pb = np.asarray(proj_b, dtype=np.float32).reshape(1, C).copy()

    in_maps = []
    for i in range(NCORES):
        in_maps.append(
            {
                "xT": np.ascontiguousarray(xT[:, i * NL : (i + 1) * NL]),
                "qkv_wT": qkv_wT,
                "qkv_b_qk": bqk,
                "qkv_b_v": bv,
                "proj_wT": pwT,
                "proj_b": pb,
            }
        )
    return in_maps


def kernel(x, qkv_w, qkv_b, proj_w, proj_b):
    nc = _get_program()
    in_maps = _host_prep(x, qkv_w, qkv_b, proj_w, proj_b)
    kw = {}
    if os.environ.get("KERNEL_TRACE_DIR"):
        kw["tmpdir"] = os.environ["KERNEL_TRACE_DIR"]
    res = run_bass_kernel_spmd(
        nc,
        in_maps,
        core_ids=list(range(NCORES)),
        trace=bool(int(os.environ.get("KERNEL_TRACE", "0"))),
        **kw,
    )
    if res.exec_time_ns is not None:
        print(f"HW exec time: {res.exec_time_ns} ns", file=sys.stderr)
    out = np.concatenate(
        [np.asarray(res.results[i]["out"]) for i in range(NCORES)], axis=0
    )
    return out.reshape(1, N, C).astype(np.float32)


if __name__ == "__main__":
    rng = np.random.default_rng(0)
    x = rng.standard_normal((1, N, C), dtype=np.float32)
    qkv_w = (rng.standard_normal((3 * C, C)) * 0.01).astype(np.float32)
    qkv_b = np.zeros((3 * C,), np.float32)
    proj_w = (rng.standard_normal((C, C)) * 0.01).astype(np.float32)
    proj_b = np.zeros((C,), np.float32)
    out = kernel(x=x, qkv_w=qkv_w, qkv_b=qkv_b, proj_w=proj_w, proj_b=proj_b)
    print(out.shape, out.dtype)
